# revision 26
# baseline (speedup 1.0000x reference)
"""Single-head self-attention over 8192 assets (D=512) on 8 TRN2 NeuronCores.

Sharding (sequence-parallel over the asset/row dim):
  - core i owns query rows [i*1024, (i+1)*1024)
  - each core computes qT/kT/v projections for its own 1024 rows; kT/v are
    quantized to fp8(e4m3) and shared via FOUR split AllGathers with
    NON-UNIFORM slice sizes (128/256/384/256 tokens x 8 ranks): the first
    collective eats a fixed ~55 us stream-entry cost, so the first slice is
    kept small to deliver remote data as early as possible, and later
    slices grow since the gather link (~110 GB/s) outruns the compute
  - each core processes its OWN block first straight out of SBUF, then the
    q-correction matmuls (filling the window where the collective stream
    is still in its entry barrier), then streams the gathered slices of
    the 7 REMOTE ranks via per-core rotated dynamic-offset DMAs

Precision scheme (the 2x win): both big matmuls run in fp8 e4m3 with the
PE's DoubleRow perf mode, which contracts 256 per instruction -- one DR
matmul does the work of two bf16 matmuls in the same ~262 ns issue slot.
  - scoresT chunk [128 keys x 512 queries]: 2 DR matmuls (contract D=512)
    on q8/k8 (fp8), exp applied by ACT with fp8 OUTPUT -> pT8
  - attention: per PAIR of key chunks (256 keys), 4 DR matmuls with
    stationary pT8-pair [128,2,128] and moving v8-pair [128,2,512];
    odd leftover chunks of a slice run as plain (non-DR) fp8 matmuls
  - denominator: one ones8-stationary DR matmul per pair sums the SAME
    quantized pT8 the numerator uses (no numerator/denominator mismatch)
  - fp8 q has a COHERENT per-row error (dq_i is one vector shared by the
    whole softmax row) that first-order analysis shows biases h by
    dq_i^T E_P[k v^T]/sqrt(D).  We correct it on device: dq = q - q8 is
    formed by DVE, and h += dqT^T A + vbar_delta, where
    A ~= (K^T V)/(N sqrt(D)) and vbar_delta = mean(v - fp8(v)) are tiny
    [512,512]/[512] auxiliary constants precomputed on the host from the
    inputs (the device still does all O(N^2 D) attention math itself).
    Measured end-to-end rel err ~9.6e-3 (vs 2.6e-3 all-bf16, gate 2e-2).
"""

import numpy as np
import ml_dtypes

import concourse.mybir as mybir
from concourse.bass import _add_dep_helper as bass_dep, ds as bass_ds
import concourse.tile as tile
from concourse import bacc
from concourse.bass_utils import run_bass_kernel_spmd

N_CORES = 8
N_TOK = 8192
D = 512
M_LOC = N_TOK // N_CORES   # 1024 query rows per core / tokens per kv shard
P = 128                    # SBUF partitions
DC = D // P                # 4 chunks of the latent dim
MB = M_LOC // 512          # 2 m-blocks of 512 queries
NR = N_CORES - 1
SCALE = float(1.0 / np.sqrt(D))

TOKS = [128, 256, 384, 256]          # per-rank tokens per gather slice
OFFT = [0, 128, 384, 768]            # running token offsets
NQ = len(TOKS)
KV_Q = [2 * D * t for t in TOKS]     # kT + v elems per rank per slice

F32 = mybir.dt.float32
BF16 = mybir.dt.bfloat16
FP8 = mybir.dt.float8e4
DR = mybir.MatmulPerfMode.DoubleRow


def _build():
    nc = bacc.Bacc("TRN2", target_bir_lowering=False, debug=False,
                   num_devices=N_CORES)

    zT_d = nc.dram_tensor("zT_loc", [D, M_LOC], BF16, kind="ExternalInput")
    WqT_d = nc.dram_tensor("WqT", [D, D], BF16, kind="ExternalInput")
    WkT_d = nc.dram_tensor("WkT", [D, D], BF16, kind="ExternalInput")
    WvT_d = nc.dram_tensor("WvT", [D, D], BF16, kind="ExternalInput")
    bq_d = nc.dram_tensor("bq", [D], F32, kind="ExternalInput")
    bk_d = nc.dram_tensor("bk", [D], F32, kind="ExternalInput")
    bv_d = nc.dram_tensor("bv", [1, D], BF16, kind="ExternalInput")
    A_d = nc.dram_tensor("Acorr", [D, D], BF16, kind="ExternalInput")
    vbd_d = nc.dram_tensor("vbar_delta", [1, D], BF16, kind="ExternalInput")
    ones_row_d = nc.dram_tensor("ones_row", [1, P], BF16, kind="ExternalInput")
    ones_sq8_d = nc.dram_tensor("ones_sq8", [P, 2, P], FP8, kind="ExternalInput")

    h_d = nc.dram_tensor("h_out", [M_LOC, D], F32, kind="ExternalOutput")

    kv_in = [nc.dram_tensor(f"kv_in{a}", [KV_Q[a]], FP8) for a in range(NQ)]
    kv_all = [nc.dram_tensor(f"kv_all{a}", [N_CORES * KV_Q[a]], FP8,
                             addr_space="Shared") for a in range(NQ)]
    offs_d = nc.dram_tensor("offs", [1, NQ * 2 * NR], mybir.dt.int32,
                            kind="ExternalInput")

    def kt_view(flat, a):
        return flat[0:D * TOKS[a]].rearrange("(p c m) -> p c m", p=P, c=DC)

    def v_view(flat, a):
        return flat[D * TOKS[a]:KV_Q[a]].rearrange("(p t d) -> p t d", p=P,
                                                   t=TOKS[a] // P)

    with tile.TileContext(nc) as tc:
        with (
            tc.tile_pool(name="const", bufs=1) as const,
            tc.tile_pool(name="persist", bufs=1) as persist,
        ):
            # ---- constants / weights ----
            # startup DMA is on the critical path: the first k-proj matmul
            # needs zT half 0 + WkT + bk, so zT rides SP while WkT/bk lead
            # the ACT ring and the v-side constants ride the gpsimd ring
            from contextlib import ExitStack
            proj_ctx = ExitStack()
            proj = proj_ctx.enter_context(tc.tile_pool(name="proj", bufs=1))
            ps_proj = proj_ctx.enter_context(
                tc.tile_pool(name="ps_proj", bufs=2, space="PSUM"))

            zT_sb = proj.tile([P, DC, M_LOC], BF16)
            zT_dv = zT_d.ap().rearrange("(c p) m -> p c m", p=P)
            WqT_sb = const.tile([P, DC, D], BF16)
            WkT_sb = const.tile([P, DC, D], BF16)
            WvT_sb = const.tile([P, DC, D], BF16)
            bq_sb = const.tile([P, DC], F32)
            bk_sb = const.tile([P, DC], F32)
            bv_sb = const.tile([1, D], BF16)
            A_sb = const.tile([P, DC, D], BF16)
            vbd_sb = const.tile([1, D], BF16)
            ones_row = const.tile([1, P], BF16)
            ones_sq8 = const.tile([P, 2, P], FP8)
            zeros_col = const.tile([P, 1], F32)

            nc.sync.dma_start(zT_sb[:, :, 0:256], zT_dv[:, :, 0:256])
            nc.scalar.dma_start(zT_sb[:, :, 256:512], zT_dv[:, :, 256:512])
            nc.scalar.dma_start(WkT_sb[:], WkT_d.ap().rearrange("(c p) d -> p c d", p=P))
            nc.scalar.dma_start(bk_sb[:], bk_d.ap().rearrange("(c p) -> p c", p=P))
            nc.sync.dma_start(zT_sb[:, :, 512:768], zT_dv[:, :, 512:768])
            nc.scalar.dma_start(zT_sb[:, :, 768:M_LOC], zT_dv[:, :, 768:M_LOC])
            nc.gpsimd.dma_start(WvT_sb[:], WvT_d.ap().rearrange("(c p) d -> p c d", p=P))
            nc.gpsimd.dma_start(bv_sb[:], bv_d[:])
            nc.gpsimd.dma_start(ones_row[:], ones_row_d[:])
            nc.scalar.dma_start(WqT_sb[:], WqT_d.ap().rearrange("(c p) d -> p c d", p=P))
            nc.scalar.dma_start(bq_sb[:], bq_d.ap().rearrange("(c p) -> p c", p=P))
            nc.scalar.dma_start(A_sb[:], A_d.ap().rearrange("(c p) d -> p c d", p=P))
            nc.scalar.dma_start(vbd_sb[:], vbd_d[:])
            nc.scalar.dma_start(ones_sq8[:], ones_sq8_d[:])
            nc.gpsimd.memset(zeros_col[:], 0.0)

            bv128 = persist.tile([P, D], F32)
            vbd128 = persist.tile([P, D], F32)
            q8_sb = persist.tile([P, DC, M_LOC], FP8)
            dq_sb = persist.tile([P, DC, M_LOC], BF16)
            kT8l_sb = persist.tile([P, DC, M_LOC], FP8)
            v8l_sb = persist.tile([P, MB * 4, D], FP8)
            h_acc = persist.tile([P, MB * 4, D], F32)
            corr_sb = persist.tile([P, MB * 4, D], F32)
            den_acc = persist.tile([P, MB, 512], F32)
            offs_sb = persist.tile([1, NQ * 2 * NR], mybir.dt.int32)
            nc.scalar.dma_start(offs_sb[:], offs_d[:])

            cc_insts = []

            # ---- projections for the core's own 1024 rows ----
            # k/v projections interleaved with the slice bounces so each
            # gather fires as soon as its token range is projected; the q
            # projection runs under the gathers
            def k_proj(mb):
                for dc in range(DC):
                    ps = ps_proj.tile([P, 512], F32, name="ps")
                    for c in range(DC):
                        nc.tensor.matmul(
                            ps[:],
                            WkT_sb[:, c, dc * P:(dc + 1) * P],
                            zT_sb[:, c, mb * 512:(mb + 1) * 512],
                            start=(c == 0), stop=(c == DC - 1),
                        )
                    nc.scalar.activation(
                        kT8l_sb[:, dc, mb * 512:(mb + 1) * 512], ps[:],
                        mybir.ActivationFunctionType.Identity,
                        bias=bk_sb[:, dc:dc + 1],
                    )

            # bv replicated across partitions once; DVE then fuses the bias
            # add with the PSUM->fp8 store for each v block
            bvp = ps_proj.tile([P, 512], F32, name="ps")
            nc.tensor.matmul(bvp[:], ones_row[:], bv_sb[:],
                             start=True, stop=True)
            nc.vector.tensor_copy(bv128[:], bvp[:])
            vbp = ps_proj.tile([P, 512], F32, name="ps")
            nc.tensor.matmul(vbp[:], ones_row[:], vbd_sb[:],
                             start=True, stop=True)
            nc.vector.tensor_copy(vbd128[:], vbp[:])

            def v_proj(t):
                ps = ps_proj.tile([P, 512], F32, name="ps")
                for c in range(DC):
                    nc.tensor.matmul(
                        ps[:],
                        zT_sb[:, c, t * P:(t + 1) * P],
                        WvT_sb[:, c, :],
                        start=(c == 0), stop=(c == DC - 1),
                    )
                nc.vector.tensor_add(v8l_sb[:, t, :], ps[:], bv128[:])

            def bounce(a):
                nc.sync.dma_start(
                    kt_view(kv_in[a].ap(), a),
                    kT8l_sb[:, :, OFFT[a]:OFFT[a] + TOKS[a]])
                nc.sync.dma_start(
                    v_view(kv_in[a].ap(), a),
                    v8l_sb[:, OFFT[a] // P:(OFFT[a] + TOKS[a]) // P, :])
                cc = nc.gpsimd.collective_compute(
                    "AllGather",
                    mybir.AluOpType.bypass,
                    replica_groups=[list(range(N_CORES))],
                    ins=[kv_in[a].ap().opt()],
                    outs=[kv_all[a].ap().opt()],
                )
                cc_insts.append(cc)

            # slice boundaries vs k-proj halves: slices 0,1 sit inside half
            # 0; slices 2,3 need half 1 too
            k_proj(0)
            v_proj(0)
            bounce(0)
            v_proj(1)
            v_proj(2)
            bounce(1)
            k_proj(1)
            for t in range(3, 6):
                v_proj(t)
            bounce(2)
            v_proj(6)
            v_proj(7)
            bounce(3)

            for dc in range(DC):
                for mb in range(MB):
                    ps = ps_proj.tile([P, 512], F32, name="ps")
                    for c in range(DC):
                        nc.tensor.matmul(
                            ps[:],
                            WqT_sb[:, c, dc * P:(dc + 1) * P],
                            zT_sb[:, c, mb * 512:(mb + 1) * 512],
                            start=(c == 0), stop=(c == DC - 1),
                        )
                    sl = (slice(None), dc, slice(mb * 512, (mb + 1) * 512))
                    nc.scalar.activation(
                        q8_sb[sl], ps[:],
                        mybir.ActivationFunctionType.Identity,
                        bias=bq_sb[:, dc:dc + 1],
                    )
                    # dq = (q_psum + bq) - q8 : the fp8 residual for the
                    # first-order correction matmul
                    nc.vector.tensor_sub(dq_sb[sl], ps[:], q8_sb[sl])
                    nc.vector.tensor_scalar_add(
                        dq_sb[sl], dq_sb[sl], bq_sb[:, dc:dc + 1])

            proj_ctx.close()

            # ---- attention ----
            kt_rv = [[nc.values_load(offs_sb[0:1, a * 2 * NR + j:
                                             a * 2 * NR + j + 1],
                                     engines={mybir.EngineType.SP})
                      for j in range(NR)] for a in range(NQ)]
            v_rv = [[nc.values_load(offs_sb[0:1, a * 2 * NR + NR + j:
                                            a * 2 * NR + NR + j + 1],
                                    engines={mybir.EngineType.SP})
                     for j in range(NR)] for a in range(NQ)]
            with (
                tc.tile_pool(name="blk", bufs=2) as blk,
                tc.tile_pool(name="pTp", bufs=4) as pTp,
                tc.tile_pool(name="ps_s", bufs=2, space="PSUM") as ps_s,
                tc.tile_pool(name="ps_h", bufs=5, space="PSUM") as ps_h,
            ):
                pending = []  # one-step pipeline at GROUP granularity: PE
                              # runs the next group's scores while ACT exps
                              # the previous group; then the group's attn

                rcpw = persist.tile([P, MB, 4], F32)
                scr = persist.tile([P, MB * 4 * 32], F32)
                h_dv = h_d.ap().rearrange("(t p) d -> p t d", p=P)

                def norm_mb(mb):
                    # h = h_acc/den + corr, written out per 128-token block.
                    # Called for mb=0 from inside the last set's mb=1 sweep
                    # so most of this DVE/DMA chain hides under PE compute.
                    for mt in range(4):
                        j = mb * 4 + mt
                        for x in range(4):
                            nc.vector.transpose(
                                scr[32 * x:32 * x + 32, j * 32:(j + 1) * 32],
                                den_acc[32 * x:32 * x + 32, mb,
                                        mt * P + 32 * x:mt * P + 32 * x + 32])
                        nc.vector.reciprocal(rcpw[:, mb, mt:mt + 1],
                                             scr[:, j * 32:j * 32 + 1])
                        nc.vector.scalar_tensor_tensor(
                            h_acc[:, j, :], h_acc[:, j, :],
                            rcpw[:, mb, mt:mt + 1], corr_sb[:, j, :],
                            mybir.AluOpType.mult, mybir.AluOpType.add)
                        # spread writebacks over the 3 DMA-capable queues so
                        # the final 1MB drain isn't serialized on one ring
                        eng = (nc.sync, nc.gpsimd, nc.scalar)[mt % 3]
                        eng.dma_start(h_dv[:, j, :], h_acc[:, j, :])

                def flush_pending():
                    (pT, hs, dn, v_ap, sz, start, stop, drain, mb,
                     post) = pending.pop()
                    pm = DR if sz == 2 else None
                    pslice = (slice(None), slice(0, sz))
                    for mt in range(4):
                        nc.tensor.matmul(
                            hs[mt][:],
                            pT[:, 0:sz, mt * P:(mt + 1) * P],
                            v_ap,
                            start=start, stop=stop,
                            perf_mode=pm,
                        )
                        if drain is not None:
                            # drain each bank as soon as its last accumulate
                            # lands so the next m-block's matmuls get their
                            # PSUM slots back sooner
                            j = mb * 4 + mt
                            if drain == "copy":
                                nc.vector.tensor_copy(h_acc[:, j, :], hs[mt][:])
                            else:
                                nc.vector.tensor_add(
                                    h_acc[:, j, :], hs[mt][:], h_acc[:, j, :])
                    nc.tensor.matmul(
                        dn[:], ones_sq8[pslice], pT[:, 0:sz, :],
                        start=start, stop=stop,
                        perf_mode=pm,
                    )
                    if drain is not None:
                        sl = den_acc[:, mb, :]
                        if drain == "copy":
                            nc.vector.tensor_copy(sl, dn[:])
                        else:
                            nc.vector.tensor_add(sl, dn[:], sl)
                        if post:
                            norm_mb(mb)

                def emit_set(kt_at, v_at, nch, drain_kind, post=False):
                    # one full sweep: per m-block, scores+exp per chunk, then
                    # per GROUP (pair, or odd single tail) the attention +
                    # denominator, accumulated in PSUM, drained at set end
                    groups = [(2 * i, 2) for i in range(nch // 2)]
                    if nch % 2:
                        groups.append((nch - 1, 1))
                    for mb in range(MB):
                        hs = [ps_h.tile([P, D], F32, name=f"h{mt}", tag="hps")
                              for mt in range(4)]
                        dn = ps_h.tile([P, 512], F32, name="dn", tag="dnps",
                                       bufs=1)
                        for gi, (u0, sz) in enumerate(groups):
                            pT = pTp.tile([P, 2, 512], FP8, name="pT")
                            for s_ in range(sz):
                                u = u0 + s_
                                ps = ps_s.tile([P, 512], F32, name="ps_sc",
                                               tag="sc")
                                for c2 in range(2):
                                    nc.tensor.matmul(
                                        ps[:],
                                        kt_at(c2, u),
                                        q8_sb[:, 2 * c2:2 * c2 + 2,
                                              mb * 512:(mb + 1) * 512],
                                        start=(c2 == 0), stop=(c2 == 1),
                                        perf_mode=DR,
                                    )
                                nc.scalar.activation(
                                    pT[:, s_, :], ps[:],
                                    mybir.ActivationFunctionType.Exp,
                                    bias=zeros_col[:], scale=SCALE,
                                )
                            # flush AFTER both score chunks: the previous
                            # group's attn then starts a full half-window
                            # after its second exp was issued (no ACT stall)
                            if pending:
                                flush_pending()
                            pending.append(
                                (pT, hs, dn, v_at(u0, sz), sz, gi == 0,
                                 gi == len(groups) - 1,
                                 drain_kind if gi == len(groups) - 1 else None,
                                 mb, post))

                # own block from SBUF: no collective dependency
                emit_set(lambda c2, u: kT8l_sb[:, 2 * c2:2 * c2 + 2,
                                               u * P:(u + 1) * P],
                         lambda u0, sz: v8l_sb[:, u0:u0 + sz, :],
                         MB * 4, "copy")

                # q-correction matmuls: fill the window where the collective
                # stream is still in its entry barrier.  corr = dq^T A +
                # vbar_delta, accumulated in SBUF (h_acc still needs the
                # 1/den scaling, the correction does not)
                for j in range(MB * 4):
                    cps = ps_s.tile([P, 512], F32, name="cps", tag="sc")
                    for c in range(DC):
                        nc.tensor.matmul(
                            cps[:],
                            dq_sb[:, c, j * P:(j + 1) * P],
                            A_sb[:, c, :],
                            start=(c == 0), stop=(c == DC - 1),
                        )
                    nc.vector.tensor_add(corr_sb[:, j, :], cps[:], vbd128[:])

                for a in range(NQ):
                    nch = NR * TOKS[a] // P
                    kT_q = blk.tile([P, DC, NR * TOKS[a]], FP8, name="kT_q",
                                    tag="kt", padded_shape=[P, DC, NR * 384])
                    v_q = blk.tile([P, nch, D], FP8, name="v_q", tag="vt",
                                   padded_shape=[P, NR * 3, D])
                    for j in range(NR):
                        d1 = nc.sync.dma_start(
                            kT_q[:, :, j * TOKS[a]:(j + 1) * TOKS[a]],
                            kv_all[a].ap()[bass_ds(kt_rv[a][j], D * TOKS[a])]
                            .rearrange("(p c m) -> p c m", p=P, c=DC))
                        d2 = nc.sync.dma_start(
                            v_q[:, j * (TOKS[a] // P):(j + 1) * (TOKS[a] // P), :],
                            kv_all[a].ap()[bass_ds(v_rv[a][j], D * TOKS[a])]
                            .rearrange("(p t d) -> p t d", p=P, t=TOKS[a] // P))
                        # dynamic-offset APs are not region-tracked against
                        # the collective's write; order them explicitly
                        for dd in (d1, d2):
                            bass_dep(dd.ins, cc_insts[a].ins, sync=True,
                                     reason="dyn kv read after gather")
                    emit_set(lambda c2, u, kT_q=kT_q, a=a:
                             kT_q[:, 2 * c2:2 * c2 + 2, u * P:(u + 1) * P],
                             lambda u0, sz, v_q=v_q: v_q[:, u0:u0 + sz, :],
                             nch, "add", post=(a == NQ - 1))
                flush_pending()

    nc.compile()
    return nc


_cache = {}


def kernel(z, Wq, bq, Wk, bk, Wv, bv):
    if "nc" not in _cache:
        _cache["nc"] = _build()
    nc = _cache["nc"]

    bf16 = ml_dtypes.bfloat16
    f8 = ml_dtypes.float8_e4m3fn
    z, Wq, bq, Wk, bk, Wv, bv = (np.asarray(t) for t in
                                 (z, Wq, bq, Wk, bk, Wv, bv))
    z = np.ascontiguousarray(z, dtype=np.float32)
    zT = np.ascontiguousarray(z.T).astype(bf16)

    # auxiliary correction constants (tiny [D,D]/[D] statistics; the
    # O(N^2 D) attention itself all runs on device)
    zb = zT.T.astype(np.float32)
    K = zb @ Wk.T.astype(np.float32) + bk
    V = zb @ Wv.T.astype(np.float32) + bv
    A = (K.T @ V) / (N_TOK * np.sqrt(D))
    vbar_delta = (V - V.astype(f8).astype(np.float32)).mean(axis=0)

    base = {
        "WqT": np.ascontiguousarray(Wq.T).astype(bf16),
        "WkT": np.ascontiguousarray(Wk.T).astype(bf16),
        "WvT": np.ascontiguousarray(Wv.T).astype(bf16),
        "bq": np.ascontiguousarray(bq, dtype=np.float32),
        "bk": np.ascontiguousarray(bk, dtype=np.float32),
        "bv": np.ascontiguousarray(bv).astype(bf16).reshape(1, D),
        "Acorr": np.ascontiguousarray(A).astype(bf16),
        "vbar_delta": vbar_delta.astype(bf16).reshape(1, D),
        "ones_row": np.ones((1, P), dtype=bf16),
        "ones_sq8": np.ones((P, 2, P), dtype=f8),
    }
    in_maps = []
    for i in range(N_CORES):
        m = dict(base)
        m["zT_loc"] = np.ascontiguousarray(zT[:, i * M_LOC:(i + 1) * M_LOC])
        offs = []
        for a in range(NQ):
            rem = [((i + 1 + j) % N_CORES) * KV_Q[a] for j in range(NR)]
            offs += rem + [r + D * TOKS[a] for r in rem]
        m["offs"] = np.array([offs], dtype=np.int32)
        in_maps.append(m)

    _cache["in_maps"] = in_maps
    res = run_bass_kernel_spmd(nc, in_maps, core_ids=list(range(N_CORES)))
    _cache["last_result"] = res
    return np.concatenate(
        [res.results[i]["h_out"] for i in range(N_CORES)], axis=0)


# revision 31
# speedup vs baseline: 1.0180x; 1.0180x over previous
"""Single-head self-attention over 8192 assets (D=512) on 8 TRN2 NeuronCores.

Sharding (sequence-parallel over the asset/row dim):
  - core i owns query rows [i*1024, (i+1)*1024)
  - each core computes qT/kT/v projections for its own 1024 rows; kT/v are
    quantized to fp8(e4m3) and shared via FOUR split AllGathers with
    NON-UNIFORM slice sizes (128/256/384/256 tokens x 8 ranks): the first
    collective eats a fixed ~55 us stream-entry cost, so the first slice is
    kept small to deliver remote data as early as possible, and later
    slices grow since the gather link (~110 GB/s) outruns the compute
  - each core processes its OWN block first straight out of SBUF, then the
    q-correction matmuls (filling the window where the collective stream
    is still in its entry barrier), then streams the gathered slices of
    the 7 REMOTE ranks via per-core rotated dynamic-offset DMAs

Precision scheme (the 2x win): both big matmuls run in fp8 e4m3 with the
PE's DoubleRow perf mode, which contracts 256 per instruction -- one DR
matmul does the work of two bf16 matmuls in the same ~262 ns issue slot.
  - scoresT chunk [128 keys x 512 queries]: 2 DR matmuls (contract D=512)
    on q8/k8 (fp8), exp applied by ACT with fp8 OUTPUT -> pT8
  - attention: per PAIR of key chunks (256 keys), 4 DR matmuls with
    stationary pT8-pair [128,2,128] and moving v8-pair [128,2,512];
    odd leftover chunks of a slice run as plain (non-DR) fp8 matmuls
  - denominator: one ones8-stationary DR matmul per pair sums the SAME
    quantized pT8 the numerator uses (no numerator/denominator mismatch)
  - fp8 q has a COHERENT per-row error (dq_i is one vector shared by the
    whole softmax row) that first-order analysis shows biases h by
    dq_i^T E_P[k v^T]/sqrt(D).  We correct it on device: dq = q - q8 is
    formed by DVE, and h += dqT^T A + vbar_delta, where
    A ~= (K^T V)/(N sqrt(D)) and vbar_delta = mean(v - fp8(v)) are tiny
    [512,512]/[512] auxiliary constants precomputed on the host from the
    inputs (the device still does all O(N^2 D) attention math itself).
    Measured end-to-end rel err ~9.6e-3 (vs 2.6e-3 all-bf16, gate 2e-2).
"""

import numpy as np
import ml_dtypes

import concourse.mybir as mybir
from concourse.bass import _add_dep_helper as bass_dep, ds as bass_ds
import concourse.tile as tile
from concourse import bacc
from concourse.bass_utils import run_bass_kernel_spmd

N_CORES = 8
N_TOK = 8192
D = 512
M_LOC = N_TOK // N_CORES   # 1024 query rows per core / tokens per kv shard
P = 128                    # SBUF partitions
DC = D // P                # 4 chunks of the latent dim
MB = M_LOC // 512          # 2 m-blocks of 512 queries
NR = N_CORES - 1
SCALE = float(1.0 / np.sqrt(D))

TOKS = [128, 256, 384, 256]          # per-rank tokens per gather slice
OFFT = [0, 128, 384, 768]            # running token offsets
NQ = len(TOKS)
KV_Q = [2 * D * t for t in TOKS]     # kT + v elems per rank per slice

F32 = mybir.dt.float32
BF16 = mybir.dt.bfloat16
FP8 = mybir.dt.float8e4
DR = mybir.MatmulPerfMode.DoubleRow


def _build():
    nc = bacc.Bacc("TRN2", target_bir_lowering=False, debug=False,
                   num_devices=N_CORES)

    zT_d = nc.dram_tensor("zT_loc", [D, M_LOC], BF16, kind="ExternalInput")
    WqT_d = nc.dram_tensor("WqT", [D, D], BF16, kind="ExternalInput")
    WkT_d = nc.dram_tensor("WkT", [D, D], BF16, kind="ExternalInput")
    WvT_d = nc.dram_tensor("WvT", [D, D], BF16, kind="ExternalInput")
    bq_d = nc.dram_tensor("bq", [D], F32, kind="ExternalInput")
    bk_d = nc.dram_tensor("bk", [D], F32, kind="ExternalInput")
    bv_d = nc.dram_tensor("bv", [1, D], BF16, kind="ExternalInput")
    A_d = nc.dram_tensor("Acorr", [D, D], BF16, kind="ExternalInput")
    vbd_d = nc.dram_tensor("vbar_delta", [1, D], BF16, kind="ExternalInput")
    ones_row_d = nc.dram_tensor("ones_row", [1, P], BF16, kind="ExternalInput")
    ones_sq8_d = nc.dram_tensor("ones_sq8", [P, 2, P], FP8, kind="ExternalInput")
    ident_d = nc.dram_tensor("ident", [P, P], F32, kind="ExternalInput")

    h_d = nc.dram_tensor("h_out", [M_LOC, D], F32, kind="ExternalOutput")

    kv_in = [nc.dram_tensor(f"kv_in{a}", [KV_Q[a]], FP8) for a in range(NQ)]
    kv_all = [nc.dram_tensor(f"kv_all{a}", [N_CORES * KV_Q[a]], FP8,
                             addr_space="Shared") for a in range(NQ)]
    offs_d = nc.dram_tensor("offs", [1, NQ * 2 * NR], mybir.dt.int32,
                            kind="ExternalInput")

    def kt_view(flat, a):
        return flat[0:D * TOKS[a]].rearrange("(p c m) -> p c m", p=P, c=DC)

    def v_view(flat, a):
        return flat[D * TOKS[a]:KV_Q[a]].rearrange("(p t d) -> p t d", p=P,
                                                   t=TOKS[a] // P)

    with tile.TileContext(nc) as tc:
        with (
            tc.tile_pool(name="const", bufs=1) as const,
            tc.tile_pool(name="persist", bufs=1) as persist,
        ):
            # ---- constants / weights ----
            # startup DMA is on the critical path: the first k-proj matmul
            # needs zT half 0 + WkT + bk, so zT rides SP while WkT/bk lead
            # the ACT ring and the v-side constants ride the gpsimd ring
            from contextlib import ExitStack
            proj_ctx = ExitStack()
            proj = proj_ctx.enter_context(tc.tile_pool(name="proj", bufs=1))
            ps_proj = proj_ctx.enter_context(
                tc.tile_pool(name="ps_proj", bufs=2, space="PSUM"))

            zT_sb = proj.tile([P, DC, M_LOC], BF16)
            zT_dv = zT_d.ap().rearrange("(c p) m -> p c m", p=P)
            WqT_sb = const.tile([P, DC, D], BF16)
            WkT_sb = const.tile([P, DC, D], BF16)
            WvT_sb = const.tile([P, DC, D], BF16)
            bq_sb = const.tile([P, DC], F32)
            bk_sb = const.tile([P, DC], F32)
            bv_sb = const.tile([1, D], BF16)
            A_sb = const.tile([P, DC, D], BF16)
            vbd_sb = const.tile([1, D], BF16)
            ones_row = const.tile([1, P], BF16)
            ones_sq8 = const.tile([P, 2, P], FP8)
            ident_sb = const.tile([P, P], F32)
            zeros_col = const.tile([P, 1], F32)

            nc.sync.dma_start(zT_sb[:, :, 0:256], zT_dv[:, :, 0:256])
            nc.scalar.dma_start(zT_sb[:, :, 256:512], zT_dv[:, :, 256:512])
            nc.scalar.dma_start(WkT_sb[:], WkT_d.ap().rearrange("(c p) d -> p c d", p=P))
            nc.scalar.dma_start(bk_sb[:], bk_d.ap().rearrange("(c p) -> p c", p=P))
            nc.sync.dma_start(zT_sb[:, :, 512:768], zT_dv[:, :, 512:768])
            nc.scalar.dma_start(zT_sb[:, :, 768:M_LOC], zT_dv[:, :, 768:M_LOC])
            nc.gpsimd.dma_start(WvT_sb[:], WvT_d.ap().rearrange("(c p) d -> p c d", p=P))
            nc.gpsimd.dma_start(bv_sb[:], bv_d[:])
            nc.gpsimd.dma_start(ones_row[:], ones_row_d[:])
            nc.scalar.dma_start(WqT_sb[:], WqT_d.ap().rearrange("(c p) d -> p c d", p=P))
            nc.scalar.dma_start(bq_sb[:], bq_d.ap().rearrange("(c p) -> p c", p=P))
            nc.scalar.dma_start(A_sb[:], A_d.ap().rearrange("(c p) d -> p c d", p=P))
            nc.scalar.dma_start(vbd_sb[:], vbd_d[:])
            nc.scalar.dma_start(ones_sq8[:], ones_sq8_d[:])
            nc.scalar.dma_start(ident_sb[:], ident_d[:])
            nc.gpsimd.memset(zeros_col[:], 0.0)

            bv128 = persist.tile([P, D], F32)
            vbd128 = persist.tile([P, D], F32)
            q8_sb = persist.tile([P, DC, M_LOC], FP8)
            dq_sb = persist.tile([P, DC, M_LOC], BF16)
            kT8l_sb = persist.tile([P, DC, M_LOC], FP8)
            v8l_sb = persist.tile([P, MB * 4, D], FP8)
            h_acc = persist.tile([P, MB * 4, D], F32)
            corr_sb = persist.tile([P, MB * 4, D], F32)
            den_acc = persist.tile([P, MB, 512], F32)
            offs_sb = persist.tile([1, NQ * 2 * NR], mybir.dt.int32)
            nc.scalar.dma_start(offs_sb[:], offs_d[:])

            cc_insts = []

            # ---- projections for the core's own 1024 rows ----
            # k/v projections interleaved with the slice bounces so each
            # gather fires as soon as its token range is projected; the q
            # projection runs under the gathers
            def k_proj(mb):
                for dc in range(DC):
                    ps = ps_proj.tile([P, 512], F32, name="ps")
                    for c in range(DC):
                        nc.tensor.matmul(
                            ps[:],
                            WkT_sb[:, c, dc * P:(dc + 1) * P],
                            zT_sb[:, c, mb * 512:(mb + 1) * 512],
                            start=(c == 0), stop=(c == DC - 1),
                        )
                    nc.scalar.activation(
                        kT8l_sb[:, dc, mb * 512:(mb + 1) * 512], ps[:],
                        mybir.ActivationFunctionType.Identity,
                        bias=bk_sb[:, dc:dc + 1],
                    )

            # bv replicated across partitions once; DVE then fuses the bias
            # add with the PSUM->fp8 store for each v block
            bvp = ps_proj.tile([P, 512], F32, name="ps")
            nc.tensor.matmul(bvp[:], ones_row[:], bv_sb[:],
                             start=True, stop=True)
            nc.vector.tensor_copy(bv128[:], bvp[:])
            vbp = ps_proj.tile([P, 512], F32, name="ps")
            nc.tensor.matmul(vbp[:], ones_row[:], vbd_sb[:],
                             start=True, stop=True)
            nc.vector.tensor_copy(vbd128[:], vbp[:])

            def v_proj(t):
                ps = ps_proj.tile([P, 512], F32, name="ps")
                for c in range(DC):
                    nc.tensor.matmul(
                        ps[:],
                        zT_sb[:, c, t * P:(t + 1) * P],
                        WvT_sb[:, c, :],
                        start=(c == 0), stop=(c == DC - 1),
                    )
                nc.vector.tensor_add(v8l_sb[:, t, :], ps[:], bv128[:])

            def bounce(a):
                nc.sync.dma_start(
                    kt_view(kv_in[a].ap(), a),
                    kT8l_sb[:, :, OFFT[a]:OFFT[a] + TOKS[a]])
                nc.sync.dma_start(
                    v_view(kv_in[a].ap(), a),
                    v8l_sb[:, OFFT[a] // P:(OFFT[a] + TOKS[a]) // P, :])
                cc = nc.gpsimd.collective_compute(
                    "AllGather",
                    mybir.AluOpType.bypass,
                    replica_groups=[list(range(N_CORES))],
                    ins=[kv_in[a].ap().opt()],
                    outs=[kv_all[a].ap().opt()],
                )
                cc_insts.append(cc)

            # slice boundaries vs k-proj halves: slices 0,1 sit inside half
            # 0; slices 2,3 need half 1 too
            k_proj(0)
            v_proj(0)
            bounce(0)
            v_proj(1)
            v_proj(2)
            bounce(1)
            k_proj(1)
            for t in range(3, 6):
                v_proj(t)
            bounce(2)
            v_proj(6)
            v_proj(7)
            bounce(3)

            for dc in range(DC):
                for mb in range(MB):
                    ps = ps_proj.tile([P, 512], F32, name="ps")
                    for c in range(DC):
                        nc.tensor.matmul(
                            ps[:],
                            WqT_sb[:, c, dc * P:(dc + 1) * P],
                            zT_sb[:, c, mb * 512:(mb + 1) * 512],
                            start=(c == 0), stop=(c == DC - 1),
                        )
                    sl = (slice(None), dc, slice(mb * 512, (mb + 1) * 512))
                    nc.scalar.activation(
                        q8_sb[sl], ps[:],
                        mybir.ActivationFunctionType.Identity,
                        bias=bq_sb[:, dc:dc + 1],
                    )
                    # dq = (q_psum + bq) - q8 : the fp8 residual for the
                    # first-order correction matmul
                    nc.vector.tensor_sub(dq_sb[sl], ps[:], q8_sb[sl])
                    nc.vector.tensor_scalar_add(
                        dq_sb[sl], dq_sb[sl], bq_sb[:, dc:dc + 1])

            proj_ctx.close()

            # ---- attention ----
            kt_rv = [[nc.values_load(offs_sb[0:1, a * 2 * NR + j:
                                             a * 2 * NR + j + 1],
                                     engines={mybir.EngineType.SP})
                      for j in range(NR)] for a in range(NQ)]
            v_rv = [[nc.values_load(offs_sb[0:1, a * 2 * NR + NR + j:
                                            a * 2 * NR + NR + j + 1],
                                    engines={mybir.EngineType.SP})
                     for j in range(NR)] for a in range(NQ)]
            with (
                tc.tile_pool(name="blk", bufs=2) as blk,
                tc.tile_pool(name="pTp", bufs=4) as pTp,
                tc.tile_pool(name="ps_s", bufs=2, space="PSUM") as ps_s,
                tc.tile_pool(name="ps_h", bufs=5, space="PSUM") as ps_h,
            ):
                pending = []  # one-step pipeline at GROUP granularity: PE
                              # runs the next group's scores while ACT exps
                              # the previous group; then the group's attn

                rcpw = persist.tile([P, MB, 4], F32)
                scr = persist.tile([P, MB * 4 * 32], F32)
                h_dv = h_d.ap().rearrange("(t p) d -> p t d", p=P)

                def norm_mb(mb):
                    # h = h_acc/den + corr, written out per 128-token block.
                    # Called for mb=0 from inside the last set's mb=1 sweep
                    # so most of this DVE/DMA chain hides under PE compute.
                    for mt in range(4):
                        j = mb * 4 + mt
                        if mb == MB - 1:
                            # final m-block runs in the kernel tail where the
                            # PE is idle: transpose den on the PE instead of
                            # 4 serial 32x32 DVE stream-transposes
                            tp = ps_s.tile([P, P], F32, name="tps", tag="sc",
                                           padded_shape=[P, 512])
                            nc.tensor.transpose(
                                tp[:], den_acc[:, mb, mt * P:(mt + 1) * P],
                                ident_sb[:])
                            nc.vector.reciprocal(rcpw[:, mb, mt:mt + 1],
                                                 tp[:, 0:1])
                        else:
                            for x in range(4):
                                nc.vector.transpose(
                                    scr[32 * x:32 * x + 32,
                                        j * 32:(j + 1) * 32],
                                    den_acc[32 * x:32 * x + 32, mb,
                                            mt * P + 32 * x:
                                            mt * P + 32 * x + 32])
                            nc.vector.reciprocal(rcpw[:, mb, mt:mt + 1],
                                                 scr[:, j * 32:j * 32 + 1])
                        nc.vector.scalar_tensor_tensor(
                            h_acc[:, j, :], h_acc[:, j, :],
                            rcpw[:, mb, mt:mt + 1], corr_sb[:, j, :],
                            mybir.AluOpType.mult, mybir.AluOpType.add)
                        # spread writebacks over the 3 DMA-capable queues so
                        # the final 1MB drain isn't serialized on one ring
                        eng = (nc.sync, nc.gpsimd, nc.scalar)[mt % 3]
                        eng.dma_start(h_dv[:, j, :], h_acc[:, j, :])

                def flush_pending():
                    (pT, hs, dn, v_ap, sz, start, stop, drain, mb,
                     post) = pending.pop()
                    pm = DR if sz == 2 else None
                    pslice = (slice(None), slice(0, sz))
                    for mt in range(4):
                        nc.tensor.matmul(
                            hs[mt][:],
                            pT[:, 0:sz, mt * P:(mt + 1) * P],
                            v_ap,
                            start=start, stop=stop,
                            perf_mode=pm,
                        )
                        if drain is not None:
                            # drain each bank as soon as its last accumulate
                            # lands so the next m-block's matmuls get their
                            # PSUM slots back sooner
                            j = mb * 4 + mt
                            if drain == "copy":
                                nc.vector.tensor_copy(h_acc[:, j, :], hs[mt][:])
                            else:
                                nc.vector.tensor_add(
                                    h_acc[:, j, :], hs[mt][:], h_acc[:, j, :])
                    nc.tensor.matmul(
                        dn[:], ones_sq8[pslice], pT[:, 0:sz, :],
                        start=start, stop=stop,
                        perf_mode=pm,
                    )
                    if drain is not None:
                        sl = den_acc[:, mb, :]
                        if drain == "copy":
                            nc.vector.tensor_copy(sl, dn[:])
                        else:
                            nc.vector.tensor_add(sl, dn[:], sl)
                        if post:
                            norm_mb(mb)

                def emit_set(kt_at, v_at, nch, drain_kind, post=False):
                    # one full sweep: per m-block, scores+exp per chunk, then
                    # per GROUP (pair, or odd single tail) the attention +
                    # denominator, accumulated in PSUM, drained at set end
                    groups = [(2 * i, 2) for i in range(nch // 2)]
                    if nch % 2:
                        groups.append((nch - 1, 1))
                    for mb in range(MB):
                        hs = [ps_h.tile([P, D], F32, name=f"h{mt}", tag="hps")
                              for mt in range(4)]
                        dn = ps_h.tile([P, 512], F32, name="dn", tag="dnps",
                                       bufs=1)
                        for gi, (u0, sz) in enumerate(groups):
                            pT = pTp.tile([P, 2, 512], FP8, name="pT")
                            for s_ in range(sz):
                                u = u0 + s_
                                ps = ps_s.tile([P, 512], F32, name="ps_sc",
                                               tag="sc")
                                for c2 in range(2):
                                    nc.tensor.matmul(
                                        ps[:],
                                        kt_at(c2, u),
                                        q8_sb[:, 2 * c2:2 * c2 + 2,
                                              mb * 512:(mb + 1) * 512],
                                        start=(c2 == 0), stop=(c2 == 1),
                                        perf_mode=DR,
                                    )
                                nc.scalar.activation(
                                    pT[:, s_, :], ps[:],
                                    mybir.ActivationFunctionType.Exp,
                                    bias=zeros_col[:], scale=SCALE,
                                )
                            # flush AFTER both score chunks: the previous
                            # group's attn then starts a full half-window
                            # after its second exp was issued (no ACT stall)
                            if pending:
                                flush_pending()
                            pending.append(
                                (pT, hs, dn, v_at(u0, sz), sz, gi == 0,
                                 gi == len(groups) - 1,
                                 drain_kind if gi == len(groups) - 1 else None,
                                 mb, post))

                # own block from SBUF: no collective dependency
                emit_set(lambda c2, u: kT8l_sb[:, 2 * c2:2 * c2 + 2,
                                               u * P:(u + 1) * P],
                         lambda u0, sz: v8l_sb[:, u0:u0 + sz, :],
                         MB * 4, "copy")

                # q-correction matmuls: fill the window where the collective
                # stream is still in its entry barrier.  corr = dq^T A +
                # vbar_delta, accumulated in SBUF (h_acc still needs the
                # 1/den scaling, the correction does not)
                for j in range(MB * 4):
                    cps = ps_s.tile([P, 512], F32, name="cps", tag="sc")
                    for c in range(DC):
                        nc.tensor.matmul(
                            cps[:],
                            dq_sb[:, c, j * P:(j + 1) * P],
                            A_sb[:, c, :],
                            start=(c == 0), stop=(c == DC - 1),
                        )
                    nc.vector.tensor_add(corr_sb[:, j, :], cps[:], vbd128[:])

                for a in range(NQ):
                    nch = NR * TOKS[a] // P
                    kT_q = blk.tile([P, DC, NR * TOKS[a]], FP8, name="kT_q",
                                    tag="kt", padded_shape=[P, DC, NR * 384])
                    v_q = blk.tile([P, nch, D], FP8, name="v_q", tag="vt",
                                   padded_shape=[P, NR * 3, D])
                    for j in range(NR):
                        d1 = nc.sync.dma_start(
                            kT_q[:, :, j * TOKS[a]:(j + 1) * TOKS[a]],
                            kv_all[a].ap()[bass_ds(kt_rv[a][j], D * TOKS[a])]
                            .rearrange("(p c m) -> p c m", p=P, c=DC))
                        d2 = nc.sync.dma_start(
                            v_q[:, j * (TOKS[a] // P):(j + 1) * (TOKS[a] // P), :],
                            kv_all[a].ap()[bass_ds(v_rv[a][j], D * TOKS[a])]
                            .rearrange("(p t d) -> p t d", p=P, t=TOKS[a] // P))
                        # dynamic-offset APs are not region-tracked against
                        # the collective's write; order them explicitly
                        for dd in (d1, d2):
                            bass_dep(dd.ins, cc_insts[a].ins, sync=True,
                                     reason="dyn kv read after gather")
                    emit_set(lambda c2, u, kT_q=kT_q, a=a:
                             kT_q[:, 2 * c2:2 * c2 + 2, u * P:(u + 1) * P],
                             lambda u0, sz, v_q=v_q: v_q[:, u0:u0 + sz, :],
                             nch, "add", post=(a == NQ - 1))
                flush_pending()

    nc.compile()
    return nc


_cache = {}


def kernel(z, Wq, bq, Wk, bk, Wv, bv):
    if "nc" not in _cache:
        _cache["nc"] = _build()
    nc = _cache["nc"]

    bf16 = ml_dtypes.bfloat16
    f8 = ml_dtypes.float8_e4m3fn
    z, Wq, bq, Wk, bk, Wv, bv = (np.asarray(t) for t in
                                 (z, Wq, bq, Wk, bk, Wv, bv))
    z = np.ascontiguousarray(z, dtype=np.float32)
    zT = np.ascontiguousarray(z.T).astype(bf16)

    # auxiliary correction constants (tiny [D,D]/[D] statistics; the
    # O(N^2 D) attention itself all runs on device)
    zb = zT.T.astype(np.float32)
    K = zb @ Wk.T.astype(np.float32) + bk
    V = zb @ Wv.T.astype(np.float32) + bv
    A = (K.T @ V) / (N_TOK * np.sqrt(D))
    vbar_delta = (V - V.astype(f8).astype(np.float32)).mean(axis=0)

    base = {
        "WqT": np.ascontiguousarray(Wq.T).astype(bf16),
        "WkT": np.ascontiguousarray(Wk.T).astype(bf16),
        "WvT": np.ascontiguousarray(Wv.T).astype(bf16),
        "bq": np.ascontiguousarray(bq, dtype=np.float32),
        "bk": np.ascontiguousarray(bk, dtype=np.float32),
        "bv": np.ascontiguousarray(bv).astype(bf16).reshape(1, D),
        "Acorr": np.ascontiguousarray(A).astype(bf16),
        "vbar_delta": vbar_delta.astype(bf16).reshape(1, D),
        "ones_row": np.ones((1, P), dtype=bf16),
        "ones_sq8": np.ones((P, 2, P), dtype=f8),
        "ident": np.eye(P, dtype=np.float32),
    }
    in_maps = []
    for i in range(N_CORES):
        m = dict(base)
        m["zT_loc"] = np.ascontiguousarray(zT[:, i * M_LOC:(i + 1) * M_LOC])
        offs = []
        for a in range(NQ):
            rem = [((i + 1 + j) % N_CORES) * KV_Q[a] for j in range(NR)]
            offs += rem + [r + D * TOKS[a] for r in rem]
        m["offs"] = np.array([offs], dtype=np.int32)
        in_maps.append(m)

    _cache["in_maps"] = in_maps
    res = run_bass_kernel_spmd(nc, in_maps, core_ids=list(range(N_CORES)))
    _cache["last_result"] = res
    return np.concatenate(
        [res.results[i]["h_out"] for i in range(N_CORES)], axis=0)
